# revision 3
# baseline (speedup 1.0000x reference)
"""ALiBi causal attention block on 8 TRN2 NeuronCores — wire-optimized.

Under axon the wall clock is dominated by host<->device transfers
(~35-45 MB/s aggregate), so v2 minimizes wire bytes:
 - every wire tensor is bf16;
 - each input byte is shipped exactly once and replicated on-device with
   DRAM AllGathers (x: pair-wise over head-group cores; weights: 4-wise
   over batch cores);
 - the c_proj partial-sum pair reduction runs on-device (ReduceScatter),
   so each core downloads a disjoint [1024, 576] bf16 slice of the final
   output (one full copy total);
 - mask/identity are NEFF inline constants (zero wire cost);
 - the jitted PJRT callable is built once and cached; output donation
   buffers are created on-device (jnp.zeros) instead of uploading zeros.

Sharding: core c -> (batch b = c//2, head-group g = c%2); 6 heads/group.
Math identical to v1: causal softmax without max-subtraction (logits are
small), ALiBi bias is zero on the causal region, ones-column appended to V
yields the softmax denominator from the PV matmul. All matmuls bf16 with
f32 PSUM accumulation; rel err vs f32 reference ~1e-2 < 2e-2 gate.
"""

import contextlib

import numpy as np
import ml_dtypes

import concourse.bass as bass
import concourse.mybir as mybir
import concourse.tile as tile
from concourse import bacc

B, T, C = 4, 2048, 576
H = 12               # total heads
HG = 6               # heads per core (head-group)
D = 48               # head dim
CG = HG * D          # 288 channels per group
NT = T // 128        # 16 row tiles
NB = T // 512        # 4  i-blocks of 512
TH = T // 2          # 1024 columns of xT shipped per core
WQC = 3 * CG // 4    # 216-column quarter of wqkvT shipped per core
WPC = C // 4         # 144-column quarter of wpT shipped per core
SCALE = 1.0 / float(np.sqrt(D))

F32 = mybir.dt.float32
BF16 = mybir.dt.bfloat16
BF16NP = ml_dtypes.bfloat16

# contraction chunks over C=576: 4x128 + 64
C_CHUNKS = [(0, 128), (128, 128), (256, 128), (384, 128), (512, 64)]
# contraction chunks over CG=288 for c_proj: 3x96
G_CHUNKS = [(0, 96), (96, 96), (192, 96)]

PAIR_GROUPS = [[0, 1], [2, 3], [4, 5], [6, 7]]      # head-group cores of a batch
QUAD_GROUPS = [[0, 2, 4, 6], [1, 3, 5, 7]]          # batch cores of a head-group


def build_nc():
    nc = bacc.Bacc("TRN2", target_bir_lowering=False, debug=False)

    xTh_d = nc.dram_tensor("xTh", [C, TH], BF16, kind="ExternalInput")
    wqh_d = nc.dram_tensor("wqh", [C, WQC], BF16, kind="ExternalInput")
    wph_d = nc.dram_tensor("wph", [CG, WPC], BF16, kind="ExternalInput")
    out_d = nc.dram_tensor("out", [TH, C], BF16, kind="ExternalOutput")

    mask_c = nc.inline_tensor(
        np.triu(np.ones((128, 128), np.float32)).astype(BF16NP), name="maskc"
    )
    ident_c = nc.inline_tensor(np.eye(128, dtype=BF16NP), name="identc")

    with tile.TileContext(nc) as tc:
        with (
            tc.tile_pool(name="dram", bufs=8, space="DRAM") as p_dram,
            tc.tile_pool(name="wp", bufs=3) as p_wp,
            tc.tile_pool(name="qk", bufs=12) as p_qk,
            tc.tile_pool(name="vb", bufs=16) as p_vb,
            tc.tile_pool(name="y", bufs=16) as p_y,
            tc.tile_pool(name="misc", bufs=1) as p_misc,
            tc.tile_pool(name="rs", bufs=8) as p_rs,
            tc.tile_pool(name="expt", bufs=22) as p_exp,
            tc.tile_pool(name="mm", bufs=5, space="PSUM") as p_mm,
            tc.tile_pool(name="sm", bufs=3, space="PSUM") as p_sm,
        ):
            # ---- on-device input replication: one AllGather per tensor ----
            bx_in = p_dram.tile([C, TH], BF16, tag="bxi")
            bx = p_dram.tile([2, C, TH], BF16, tag="bx")
            bwq_in = p_dram.tile([C, WQC], BF16, tag="bwqi")
            bwq = p_dram.tile([4, C, WQC], BF16, tag="bwq")
            bwp_in = p_dram.tile([CG, WPC], BF16, tag="bwpi")
            bwp = p_dram.tile([4, CG, WPC], BF16, tag="bwp")
            by = p_dram.tile([T, C], BF16, tag="by")
            brs = p_dram.tile([TH, C], BF16, tag="brs")

            nc.gpsimd.dma_start(bx_in[:], xTh_d[:, :])
            nc.gpsimd.dma_start(bwq_in[:], wqh_d[:, :])
            nc.gpsimd.dma_start(bwp_in[:], wph_d[:, :])
            nc.gpsimd.collective_compute(
                "AllGather", mybir.AluOpType.bypass,
                replica_groups=PAIR_GROUPS, ins=[bx_in.opt()], outs=[bx.opt()],
            )
            nc.gpsimd.collective_compute(
                "AllGather", mybir.AluOpType.bypass,
                replica_groups=QUAD_GROUPS, ins=[bwq_in.opt()], outs=[bwq.opt()],
            )
            nc.gpsimd.collective_compute(
                "AllGather", mybir.AluOpType.bypass,
                replica_groups=QUAD_GROUPS, ins=[bwp_in.opt()], outs=[bwp.opt()],
            )

            # ---- load constants / gathered inputs into SBUF (bf16) ----
            mask_t = p_misc.tile([128, 128], BF16, tag="mask")
            nc.sync.dma_start(mask_t[:], mask_c[:, :])
            ident_t = p_misc.tile([128, 128], BF16, tag="ident")
            nc.sync.dma_start(ident_t[:], ident_c[:, :])

            stk = contextlib.ExitStack()
            p_xt = stk.enter_context(tc.tile_pool(name="xt", bufs=5))
            p_wq = stk.enter_context(tc.tile_pool(name="wq", bufs=5))
            xt, wq = [], []
            for i, (c0, cn) in enumerate(C_CHUNKS):
                tw = p_wq.tile([128, 3 * CG], BF16, tag="wq", name="wq")
                for m in range(4):
                    nc.sync.dma_start(
                        tw[:cn, m * WQC:(m + 1) * WQC], bwq[m, c0:c0 + cn, :]
                    )
                wq.append(tw)
                t_ = p_xt.tile([128, T], BF16, tag="xt", name="xt")
                for blk in range(2):
                    nc.sync.dma_start(
                        t_[:cn, blk * TH:(blk + 1) * TH], bx[blk, c0:c0 + cn, :]
                    )
                xt.append(t_)
            wp = []
            for i, (g0, gn) in enumerate(G_CHUNKS):
                t_ = p_wp.tile([96, C], BF16, tag="wp", name="wp")
                for m in range(4):
                    nc.sync.dma_start(
                        t_[:, m * WPC:(m + 1) * WPC], bwp[m, g0:g0 + gn, :]
                    )
                wp.append(t_)

            # ---- v with ones column per head: vb tiles [128, 6*49] bf16 ----
            # qkvT col space of wq: q 0..287, k 288..575, v 576..863
            vb = []
            for it in range(NT):
                vt = p_vb.tile([128, HG * (D + 1)], BF16, tag="vb", name="vb")
                ps = p_mm.tile([128, 512], F32, tag="mm", name="mm")
                for ck, (c0, cn) in enumerate(C_CHUNKS):
                    nc.tensor.matmul(
                        ps[:, :CG],
                        xt[ck][:cn, it * 128:(it + 1) * 128],
                        wq[ck][:cn, 2 * CG:3 * CG],
                        start=(ck == 0), stop=(ck == len(C_CHUNKS) - 1),
                    )
                dst = vt[:, :].rearrange("p (h x) -> p h x", x=D + 1)
                nc.vector.tensor_copy(
                    dst[:, :, 0:D],
                    ps[:, :CG].rearrange("p (h d) -> p h d", d=D),
                )
                nc.vector.memset(dst[:, :, D:D + 1], 1.0)
                vb.append(vt)

            # ---- q,k into [64, T] bf16 tiles (head pair base partition 0) ----
            qk = []  # q0..q5, k0..k5
            for m in range(12):
                qk.append(p_qk.tile([64, T], BF16, tag="qk", name="qk"))
            for h in range(HG):
                for m in (h, 6 + h):      # q then k of head h
                    r0 = m * D
                    for ib in range(NB):
                        ps = p_mm.tile([128, 512], F32, tag="mm", name="mm")
                        for ck, (c0, cn) in enumerate(C_CHUNKS):
                            nc.tensor.matmul(
                                ps[0:D, :],
                                wq[ck][:cn, r0:r0 + D],
                                xt[ck][:cn, ib * 512:(ib + 1) * 512],
                                start=(ck == 0), stop=(ck == len(C_CHUNKS) - 1),
                            )
                        sl = slice(ib * 512, (ib + 1) * 512)
                        nc.vector.tensor_copy(qk[m][0:D, sl], ps[0:D, :])

            stk.close()  # free xt/wq SBUF for phase B pools
            stk2 = contextlib.ExitStack()
            p_yt = stk2.enter_context(tc.tile_pool(name="yt", bufs=6))
            p_osb = stk2.enter_context(tc.tile_pool(name="osb", bufs=2))

            # ---- attention per head; y tiles bf16 [128, CG] ----
            y = []
            for it in range(NT):
                y.append(p_y.tile([128, CG], BF16, tag="y", name="y"))

            for ib in range(NB):
                for h in range(HG):
                    qt = qk[h]
                    kt = qk[6 + h]
                    off = 0
                    njt = 4 * ib + 4
                    etiles = []
                    for jt in range(njt):
                        diag_o = jt - 4 * ib          # >=0: j-tile inside i-block
                        lo = max(diag_o, 0) * 128     # local col start
                        ps = p_mm.tile([128, 512], F32, tag="mm", name="mm")
                        et = p_exp.tile([128, 512], BF16, tag="expt", name="expt")
                        nc.tensor.matmul(
                            ps[:, lo:512],
                            kt[off:off + D, jt * 128:(jt + 1) * 128],
                            qt[off:off + D, ib * 512 + lo:(ib + 1) * 512],
                            start=True, stop=True,
                        )
                        nc.scalar.activation(
                            et[:, lo:512], ps[:, lo:512],
                            mybir.ActivationFunctionType.Exp, scale=SCALE,
                        )
                        if diag_o >= 0:
                            nc.vector.tensor_mul(
                                et[:, lo:lo + 128], et[:, lo:lo + 128], mask_t[:]
                            )
                        etiles.append(et)
                    for o in range(4):
                        itg = 4 * ib + o
                        yp = p_sm.tile([128, D + 1], F32, tag="sm", name="sm")
                        for jt in range(itg + 1):
                            nc.tensor.matmul(
                                yp[:, :],
                                etiles[jt][:, o * 128:(o + 1) * 128],
                                vb[jt][:, h * (D + 1):(h + 1) * (D + 1)],
                                start=(jt == 0), stop=(jt == itg),
                            )
                        rs = p_rs.tile([128, 1], F32, tag="rs", name="rs")
                        nc.vector.reciprocal(rs[:], yp[:, D:D + 1])
                        nc.vector.tensor_scalar_mul(
                            y[itg][:, h * D:(h + 1) * D], yp[:, :D], rs[:]
                        )

                # fused tail for this i-block: transpose y -> yT, c_proj,
                # DMA partial rows into the DRAM reduce buffer
                for o in range(4):
                    it = 4 * ib + o
                    ytl = []
                    for m, (g0, gn) in enumerate(G_CHUNKS):
                        tp = p_sm.tile([128, 128], BF16, tag="sm", name="tp")
                        nc.tensor.transpose(
                            tp[:96, :], y[it][:, g0:g0 + gn], ident_t[:]
                        )
                        ytt = p_yt.tile([96, 128], BF16, tag="yt", name="ytt")
                        nc.vector.tensor_copy(ytt[:, :], tp[:96, :])
                        ytl.append(ytt)
                    ob = p_osb.tile([128, C], BF16, tag="osb", name="osb")
                    for nb in range(2):
                        ps = p_sm.tile([128, CG], F32, tag="sm", name="sm")
                        for m in range(3):
                            nc.tensor.matmul(
                                ps[:, :],
                                ytl[m][:, :],
                                wp[m][:, nb * CG:(nb + 1) * CG],
                                start=(m == 0), stop=(m == 2),
                            )
                        nc.vector.tensor_copy(ob[:, nb * CG:(nb + 1) * CG], ps[:, :])
                    nc.sync.dma_start(by[it * 128:(it + 1) * 128, :], ob[:, :])

            stk2.close()

            # ---- on-device pair reduction; each core keeps a disjoint half ----
            nc.gpsimd.collective_compute(
                "ReduceScatter", mybir.AluOpType.add,
                replica_groups=PAIR_GROUPS, ins=[by.opt()], outs=[brs.opt()],
            )
            nc.gpsimd.dma_start(out_d[:, :], brs[:])

    nc.compile()
    return nc


def make_in_maps(x, w_qkv, w_proj):
    """Per-core bf16 shards; each input byte shipped exactly once."""
    xT = [np.ascontiguousarray(x[b].T).astype(BF16NP) for b in range(B)]
    wqT, wpT = [], []
    for g in range(2):
        w = np.concatenate(
            [w_qkv[s * C + g * CG:s * C + (g + 1) * CG] for s in range(3)], 0
        )  # [864, 576]
        wqT.append(np.ascontiguousarray(w.T).astype(BF16NP))          # [576, 864]
        wpT.append(
            np.ascontiguousarray(w_proj[:, g * CG:(g + 1) * CG].T).astype(BF16NP)
        )  # [288, 576]
    in_maps = []
    for c in range(8):
        b, g = c // 2, c % 2
        in_maps.append({
            "xTh": np.ascontiguousarray(xT[b][:, g * TH:(g + 1) * TH]),
            "wqh": np.ascontiguousarray(wqT[g][:, b * WQC:(b + 1) * WQC]),
            "wph": np.ascontiguousarray(wpT[g][:, b * WPC:(b + 1) * WPC]),
        })
    return in_maps


_NC_CACHE = {}


def _get_runner():
    """Build nc + a persistent jitted PJRT callable (cached)."""
    if "runner" in _NC_CACHE:
        return _NC_CACHE["runner"]

    import jax
    import jax.numpy as jnp
    from jax.sharding import Mesh, NamedSharding, PartitionSpec
    from jax.experimental.shard_map import shard_map
    from concourse import bass2jax

    nc = _NC_CACHE.get("nc")
    if nc is None:
        nc = build_nc()
        _NC_CACHE["nc"] = nc

    bass2jax.install_neuronx_cc_hook()

    n_cores = 8
    partition_name = nc.partition_id_tensor.name if nc.partition_id_tensor else None
    in_names, out_names, out_avals, out_np = [], [], [], []
    for alloc in nc.m.functions[0].allocations:
        if not isinstance(alloc, mybir.MemoryLocationSet):
            continue
        name = alloc.memorylocations[0].name
        if alloc.kind == "ExternalInput":
            if name != partition_name:
                in_names.append(name)
        elif alloc.kind == "ExternalOutput":
            shape = tuple(alloc.tensor_shape)
            dtype = mybir.dt.np(alloc.dtype)
            out_avals.append(jax.core.ShapedArray(shape, dtype))
            out_names.append(name)
            out_np.append((shape, dtype))
    n_params = len(in_names)
    n_outs = len(out_avals)
    all_in_names = list(in_names) + list(out_names)
    if partition_name is not None:
        all_in_names.append(partition_name)
    donate = tuple(range(n_params, n_params + n_outs))

    def _body(*args):
        operands = list(args)
        if partition_name is not None:
            operands.append(bass2jax.partition_id_tensor())
        outs = bass2jax._bass_exec_p.bind(
            *operands,
            out_avals=tuple(out_avals),
            in_names=tuple(all_in_names),
            out_names=tuple(out_names),
            lowering_input_output_aliases=(),
            sim_require_finite=True,
            sim_require_nnan=True,
            nc=nc,
        )
        return tuple(outs)

    devices = jax.devices()[:n_cores]
    mesh = Mesh(np.asarray(devices), ("core",))
    in_specs = (PartitionSpec("core"),) * (n_params + n_outs)
    out_specs = (PartitionSpec("core"),) * n_outs
    sharded = jax.jit(
        shard_map(_body, mesh=mesh, in_specs=in_specs, out_specs=out_specs,
                  check_rep=False),
        donate_argnums=donate,
        keep_unused=True,
    )

    shard0 = NamedSharding(mesh, PartitionSpec("core"))

    def _zeros():
        return tuple(
            jnp.zeros((n_cores * s[0], *s[1:]), d) for (s, d) in out_np
        )

    zeros_fn = jax.jit(_zeros, out_shardings=(shard0,) * n_outs)

    def run(in_maps):
        per_core = [[np.asarray(m[name]) for name in in_names] for m in in_maps]
        concat_in = [
            np.concatenate([per_core[c][i] for c in range(n_cores)], axis=0)
            for i in range(n_params)
        ]
        zeros_arrs = zeros_fn()
        out_arrs = sharded(*concat_in, *zeros_arrs)
        fetched = jax.device_get(list(out_arrs))
        return [
            {
                name: fetched[i].reshape(n_cores, *out_np[i][0])[c]
                for i, name in enumerate(out_names)
            }
            for c in range(n_cores)
        ]

    _NC_CACHE["runner"] = run
    return run


def _run(x, w_qkv, w_proj, trace=False):
    run = _get_runner()
    in_maps = make_in_maps(x, w_qkv, w_proj)
    results = run(in_maps)
    halves = [np.asarray(results[c]["out"], np.float32) for c in range(8)]
    full = np.stack(
        [np.concatenate([halves[2 * b], halves[2 * b + 1]], 0) for b in range(B)], 0
    )
    return full, results


def kernel(x, w_qkv, w_proj):
    x = np.asarray(x, np.float32)
    w_qkv = np.asarray(w_qkv, np.float32)
    w_proj = np.asarray(w_proj, np.float32)
    out, _ = _run(x, w_qkv, w_proj, trace=False)
    return out
